# revision 1
# baseline (speedup 1.0000x reference)
"""Trainium2 Bass kernel for nn_MultiModalInputEmbeddings.

Data-parallel over batch: 8 cores x 8 batch rows = 4096 tokens/core.
Token slot convention is column-major: token t <-> (partition t%128, slot
t//128), matching the dma_gather/dma_scatter_add custom-op layout.

Per core:
  - Branch logic folds into one bf16 "combined table" gather:
      ctab[0:1000]   = prop_emb + type_emb[0]     (word tokens)
      ctab[1000:1003]= type_emb[3:6]              (special tokens)
      ctab[1003]     = 0                          (smiles placeholder)
      ctab[1004]     = val_b + type_emb[2]        (value tokens)
  - Dense pass (all tokens): e = ctab-row + pos-row + mval*val_w, built by
    accumulating bf16 identity-matmuls into fp32 PSUM (no DVE adds), then
    LayerNorm straight off PSUM (bn_stats/bn_aggr + one tensor_scalar).
    Smiles rows are zeroed for free by folding (1-m_smiles) into the LN
    scale, and the whole pass is written with one plain strided DMA per
    group.
  - SMILES tokens are stream-compacted on device (cross-partition prefix
    sum via triangular matmul + per-row Hillis-Steele + indirect scatter of
    packed (token,pos) records), their fingerprints gathered compactly,
    run through the 768->3072->768 FFN in bf16 (fp32 PSUM accumulation),
    transposed back token-major into PSUM where pos rows join via identity
    matmuls, LayerNorm'd, and dma_scatter_add'ed onto the (zeroed) output
    rows. Compaction padding routes to dump rows past the real output.
"""

import sys

try:
    import concourse  # noqa: F401
except ImportError:  # pragma: no cover
    sys.path.insert(0, "/opt/trn_rl_repo")

import numpy as np

import concourse.bacc as bacc
import concourse.bass as bass  # noqa: F401
import concourse.mybir as mybir
import concourse.tile as tile
from concourse import bass_utils
from concourse.bass import IndirectOffsetOnAxis

F32 = mybir.dt.float32
BF16 = mybir.dt.bfloat16
I32 = mybir.dt.int32
I16 = mybir.dt.int16
ALU = mybir.AluOpType
ACTF = mybir.ActivationFunctionType

B, S, FP, HID = 64, 512, 768, 768
N_CORES = 8
B_LOC = B // N_CORES
N_TOK = B_LOC * S            # 4096 tokens/core
KJ = N_TOK // 128            # 32 slots per partition
NW = N_TOK // 16             # 256 wrapped-index columns
COL_VOCAB, MAX_POS = 1000, 512
H4 = 4 * FP
CTAB_ROWS = COL_VOCAB + 5
ZROW = COL_VOCAB + 3
VROW = COL_VOCAB + 4
DUMP = N_TOK                 # output dump row for compaction padding

S_BLKS = (512, 384)          # smiles capacity 896 = mean 683 + 8.9 sigma
CAP_S = sum(S_BLKS)
DG = 4                       # dense token-tiles per group
EPS = 1e-12


def _replicated_load(nc, dst, src_ap):
    """Load a [16, C] DRAM view into all 8 GPSIMD 16-partition groups."""
    for k in range(8):
        nc.sync.dma_start(out=dst[16 * k : 16 * k + 16, :], in_=src_ap)


def build_program(skip_gb: bool):
    nc = bacc.Bacc(
        "TRN2",
        target_bir_lowering=False,
        debug=False,
        enable_asserts=False,
        num_devices=N_CORES,
    )

    def din(name, shape, dt=F32):
        return nc.dram_tensor(name, shape, dt, kind="ExternalInput").ap()

    fps = din("fps", [N_TOK, FP])
    wtok = din("wtok", [N_TOK], I32)
    vals = din("vals", [N_TOK])
    ttyp = din("ttyp", [N_TOK], I32)
    posi = din("posi", [N_TOK], I32)
    fc1_w = din("fc1_w", [FP, H4])
    fc1_b = din("fc1_b", [H4])
    fc2_w = din("fc2_w", [H4, HID])
    fc2_b = din("fc2_b", [HID])
    prop = din("prop", [COL_VOCAB, HID])
    val_w = din("val_w", [HID])
    val_b = din("val_b", [HID])
    pose = din("pose", [MAX_POS, HID])
    typee = din("typee", [8, HID])
    ln_g = din("ln_g", [HID])
    ln_b = din("ln_b", [HID])
    ident_d = din("ident", [128, 128])
    identbf_d = din("identbf", [128, 128], BF16)
    lexclt_d = din("lexclt", [128, 128])
    ones_col_d = din("ones_col", [128, 1])
    ones_row_d = din("ones_row", [1, 128])
    iota_c_d = din("iota_c", [128, KJ], I32)

    out = nc.dram_tensor("out", [N_TOK + 128, HID], F32, kind="ExternalOutput").ap()
    ctab = nc.dram_tensor("ctab", [CTAB_ROWS, HID], F32, kind="Internal").ap()
    packed = nc.dram_tensor("packed", [1024, 2], I32, kind="Internal").ap()

    from contextlib import ExitStack

    with tile.TileContext(nc) as tc, ExitStack() as es:
        cpool = es.enter_context(tc.tile_pool(name="const", bufs=1))
        wpool = es.enter_context(tc.tile_pool(name="wts", bufs=1))
        spool = es.enter_context(tc.tile_pool(name="small", bufs=1))
        epool = es.enter_context(tc.tile_pool(name="emb", bufs=2))
        fpool = es.enter_context(tc.tile_pool(name="ffn", bufs=1))
        ppool = es.enter_context(tc.tile_pool(name="psum", bufs=1, space="PSUM"))

        # ---- constants ----
        ident = cpool.tile([128, 128], F32)
        nc.sync.dma_start(out=ident[:], in_=ident_d[:])
        identbf = cpool.tile([128, 128], BF16)
        nc.sync.dma_start(out=identbf[:], in_=identbf_d[:])
        lexclt = cpool.tile([128, 128], F32)
        nc.sync.dma_start(out=lexclt[:], in_=lexclt_d[:])
        ones_col = cpool.tile([128, 1], F32)
        nc.sync.dma_start(out=ones_col[:], in_=ones_col_d[:])
        ones_row = cpool.tile([1, 128], F32)
        nc.sync.dma_start(out=ones_row[:], in_=ones_row_d[:])
        iota_c = cpool.tile([128, KJ], I32)
        nc.sync.dma_start(out=iota_c[:], in_=iota_c_d[:])
        eps_t = cpool.tile([128, 1], F32)
        nc.vector.memset(eps_t[:], EPS)

        # ---- weights (bf16 via SWDGE cast-load) ----
        w1 = wpool.tile([128, FP // 128, H4], BF16)
        nc.gpsimd.dma_start(out=w1[:], in_=fc1_w.rearrange("(k p) m -> p k m", p=128))
        w2 = wpool.tile([128, H4 // 128, HID], BF16)
        nc.gpsimd.dma_start(out=w2[:], in_=fc2_w.rearrange("(k p) m -> p k m", p=128))
        b1 = cpool.tile([128, H4 // 128], F32)
        nc.sync.dma_start(out=b1[:], in_=fc1_b.rearrange("(m p) -> p m", p=128))
        b2 = cpool.tile([128, HID // 128], F32)
        nc.sync.dma_start(out=b2[:], in_=fc2_b.rearrange("(m p) -> p m", p=128))
        t1pm = spool.tile([128, HID // 128], F32, tag="t1pm")
        nc.sync.dma_start(out=t1pm[:], in_=typee[1, :].rearrange("(m p) -> p m", p=128))
        nc.vector.tensor_tensor(out=b2[:], in0=b2[:], in1=t1pm[:], op=ALU.add)

        t0b = cpool.tile([128, HID], F32)
        nc.sync.dma_start(out=t0b[:], in_=typee[0:1, :].to_broadcast([128, HID]))
        vwb = cpool.tile([128, HID], F32)
        nc.sync.dma_start(out=vwb[:], in_=val_w[None, :].to_broadcast([128, HID]))
        vbrow = cpool.tile([1, HID], F32)
        nc.sync.dma_start(out=vbrow[:], in_=val_b[None, :])
        t2row = spool.tile([1, HID], F32, tag="t2row")
        nc.sync.dma_start(out=t2row[:], in_=typee[2:3, :])
        nc.vector.tensor_tensor(out=vbrow[:], in0=vbrow[:], in1=t2row[:], op=ALU.add)

        if not skip_gb:
            gb = cpool.tile([128, HID], F32)
            nc.sync.dma_start(out=gb[:], in_=ln_g[None, :].to_broadcast([128, HID]))
            bb = cpool.tile([128, HID], F32)
            nc.sync.dma_start(out=bb[:], in_=ln_b[None, :].to_broadcast([128, HID]))

        # ---- build bf16 ctab / posebf in DRAM ----
        for c in range((COL_VOCAB + 127) // 128):
            r0, r1 = c * 128, min(c * 128 + 128, COL_VOCAB)
            nrow = r1 - r0
            ch = spool.tile([128, HID], F32, tag="ctab_ch")
            nc.sync.dma_start(out=ch[:nrow], in_=prop[r0:r1, :])
            nc.vector.tensor_tensor(out=ch[:nrow], in0=ch[:nrow], in1=t0b[:nrow], op=ALU.add)
            nc.sync.dma_start(out=ctab[r0:r1, :], in_=ch[:nrow])
        chsp = spool.tile([5, HID], F32, tag="chsp")
        nc.vector.memset(chsp[:], 0.0)
        nc.sync.dma_start(out=chsp[0:3, :], in_=typee[3:6, :])
        nc.sync.dma_start(out=chsp[4:5, :], in_=vbrow[:])
        nc.sync.dma_start(out=ctab[COL_VOCAB:CTAB_ROWS, :], in_=chsp[:])

        # ---- wrapped (replicated x8) index tiles ----
        tt_w = cpool.tile([128, NW], I32)
        _replicated_load(nc, tt_w, ttyp.rearrange("(s p) -> p s", p=16))
        wt_w = spool.tile([128, NW], I32, tag="wt_w")
        _replicated_load(nc, wt_w, wtok.rearrange("(s p) -> p s", p=16))
        pos_w32 = spool.tile([128, NW], I32, tag="pos_w32")
        _replicated_load(nc, pos_w32, posi.rearrange("(s p) -> p s", p=16))
        pos16 = cpool.tile([128, NW], I16)
        nc.vector.tensor_copy(pos16[:], pos_w32[:])

        m_w_w = spool.tile([128, NW], I32, tag="m_w_w")
        nc.vector.tensor_scalar(m_w_w[:], tt_w[:], 0, None, ALU.is_equal)
        m_sp_w = spool.tile([128, NW], I32, tag="m_sp_w")
        nc.vector.tensor_scalar(m_sp_w[:], tt_w[:], 3, None, ALU.is_ge)
        m_v_w = spool.tile([128, NW], I32, tag="m_v_w")
        nc.vector.tensor_scalar(m_v_w[:], tt_w[:], 2, None, ALU.is_equal)
        cidx32 = spool.tile([128, NW], I32, tag="cidx32")
        nc.vector.memset(cidx32[:], ZROW)
        nc.vector.copy_predicated(cidx32[:], m_w_w[:], wt_w[:])
        tspec = spool.tile([128, NW], I32, tag="tspec")
        nc.vector.tensor_scalar(tspec[:], tt_w[:], COL_VOCAB - 3, None, ALU.add)
        nc.vector.copy_predicated(cidx32[:], m_sp_w[:], tspec[:])
        vrow_w = spool.tile([128, NW], I32, tag="vrow_w")
        nc.vector.memset(vrow_w[:], VROW)
        nc.vector.copy_predicated(cidx32[:], m_v_w[:], vrow_w[:])
        cidx16 = cpool.tile([128, NW], I16)
        nc.vector.tensor_copy(cidx16[:], cidx32[:])

        # ---- column-major per-token tiles ----
        tt_c = cpool.tile([128, KJ], I32)
        nc.sync.dma_start(out=tt_c[:], in_=ttyp.rearrange("(j p) -> p j", p=128))
        va_c = spool.tile([128, KJ], F32, tag="va_c")
        nc.sync.dma_start(out=va_c[:], in_=vals.rearrange("(j p) -> p j", p=128))
        pos_c = spool.tile([128, KJ], I32, tag="pos_c")
        nc.sync.dma_start(out=pos_c[:], in_=posi.rearrange("(j p) -> p j", p=128))

        m_s_ci = cpool.tile([128, KJ], I32)
        nc.vector.tensor_scalar(m_s_ci[:], tt_c[:], 1, None, ALU.is_equal)
        m_ns_ci = spool.tile([128, KJ], I32, tag="m_ns_ci")
        nc.vector.tensor_scalar(m_ns_ci[:], tt_c[:], 1, None, ALU.not_equal)
        m_s_c = cpool.tile([128, KJ], F32)
        nc.vector.tensor_copy(m_s_c[:], m_s_ci[:])
        notm_c = cpool.tile([128, KJ], F32)
        nc.vector.tensor_scalar(notm_c[:], m_s_c[:], -1.0, 1.0, ALU.mult, ALU.add)
        m_v_ci = spool.tile([128, KJ], I32, tag="m_v_ci")
        nc.vector.tensor_scalar(m_v_ci[:], tt_c[:], 2, None, ALU.is_equal)
        m_v_cf = spool.tile([128, KJ], F32, tag="m_v_cf")
        nc.vector.tensor_copy(m_v_cf[:], m_v_ci[:])
        mval_c = cpool.tile([128, KJ], F32)
        nc.vector.tensor_tensor(out=mval_c[:], in0=va_c[:], in1=m_v_cf[:], op=ALU.mult)

        # ---- smiles compaction ----
        exc_ps = ppool.tile([128, KJ], F32, tag="tp")
        nc.tensor.matmul(out=exc_ps[:], lhsT=lexclt[:], rhs=m_s_c[:])
        exc = spool.tile([128, KJ], F32, tag="exc")
        nc.vector.tensor_copy(exc[:], exc_ps[:])
        cs_ps = ppool.tile([1, KJ], F32, tag="tp")
        nc.tensor.matmul(out=cs_ps[:], lhsT=ones_col[:], rhs=m_s_c[:])
        csum = spool.tile([1, KJ], F32, tag="csum")
        nc.vector.tensor_copy(csum[:], cs_ps[:])
        cur = csum
        for sh in (1, 2, 4, 8, 16):
            nxt = spool.tile([1, KJ], F32, tag=f"cs{sh}")
            nc.vector.tensor_copy(nxt[:], cur[:])
            nc.vector.tensor_tensor(
                out=nxt[:, sh:], in0=cur[:, sh:], in1=cur[:, : KJ - sh], op=ALU.add
            )
            cur = nxt
        base_row = spool.tile([1, KJ], F32, tag="base_row")
        nc.vector.tensor_tensor(out=base_row[:], in0=cur[:], in1=csum[:], op=ALU.subtract)
        bb_ps = ppool.tile([128, KJ], F32, tag="tp")
        nc.tensor.matmul(out=bb_ps[:], lhsT=ones_row[:], rhs=base_row[:])
        slot_f = spool.tile([128, KJ], F32, tag="slot_f")
        nc.vector.tensor_tensor(out=slot_f[:], in0=exc[:], in1=bb_ps[:], op=ALU.add)
        caps_t = spool.tile([128, KJ], F32, tag="caps_t")
        nc.vector.memset(caps_t[:], float(CAP_S))
        nc.vector.copy_predicated(slot_f[:], m_ns_ci[:], caps_t[:])
        slot_i = spool.tile([128, KJ], I32, tag="slot_i")
        nc.vector.tensor_copy(slot_i[:], slot_f[:])

        pack_c = spool.tile([128, KJ, 2], I32, tag="pack_c")
        nc.vector.tensor_copy(pack_c[:, :, 0], iota_c[:])
        nc.vector.tensor_copy(pack_c[:, :, 1], pos_c[:])
        pinit = spool.tile([128, 8, 2], I32, tag="pinit")
        nc.vector.memset(pinit[:], 0)
        nc.vector.memset(pinit[:, :, 0:1], DUMP)
        nc.sync.dma_start(out=packed.rearrange("(p j) t -> p j t", p=128), in_=pinit[:])
        for j in range(KJ):
            nc.gpsimd.indirect_dma_start(
                out=packed[:],
                out_offset=IndirectOffsetOnAxis(ap=slot_i[:, j : j + 1], axis=0),
                in_=pack_c[:, j, :],
                in_offset=None,
            )
        slist32 = spool.tile([128, CAP_S // 16], I32, tag="slist32")
        _replicated_load(
            nc, slist32, packed[:CAP_S, 0].rearrange("(s p) -> p s", p=16)
        )
        psm32 = spool.tile([128, CAP_S // 16], I32, tag="psm32")
        _replicated_load(
            nc, psm32, packed[:CAP_S, 1].rearrange("(s p) -> p s", p=16)
        )
        dest16 = cpool.tile([128, CAP_S // 16], I16)
        nc.vector.tensor_copy(dest16[:], slist32[:])
        sfps16 = cpool.tile([128, CAP_S // 16], I16)
        nc.vector.tensor_scalar(sfps16[:], slist32[:], N_TOK - 1, None, ALU.min)
        psm16 = cpool.tile([128, CAP_S // 16], I16)
        nc.vector.tensor_copy(psm16[:], psm32[:])

        def ln_apply(x512, x256, o768, rs, nb, zero_col=None):
            """LayerNorm apply from two PSUM halves into an SBUF f32 tile."""
            st = spool.tile([128, 2, 6], F32, tag="ln_st", bufs=3)
            mv = spool.tile([128, 2], F32, tag="ln_mv", bufs=3)
            nc.vector.bn_stats(st[:, 0, :], x512)
            nc.vector.bn_stats(st[:, 1, :], x256)
            nc.vector.bn_aggr(mv[:], st[:])
            std = spool.tile([128, 1], F32, tag="ln_std", bufs=3)
            nc.scalar.activation(std[:], mv[:, 1:2], ACTF.Sqrt, bias=eps_t[:, 0:1], scale=1.0)
            nc.vector.reciprocal(rs[:], std[:])
            if zero_col is not None and skip_gb:
                nc.vector.tensor_tensor(out=rs[:], in0=rs[:], in1=zero_col, op=ALU.mult)
            nc.vector.tensor_scalar(nb[:], mv[:, 0:1], rs[:, 0:1], -1.0, ALU.mult, ALU.mult)
            nc.vector.tensor_scalar(o768[:, 0:512], x512, rs[:, 0:1], nb[:, 0:1], ALU.mult, ALU.add)
            nc.vector.tensor_scalar(o768[:, 512:768], x256, rs[:, 0:1], nb[:, 0:1], ALU.mult, ALU.add)

        def gb_apply(o768, zero_col=None):
            if skip_gb:
                return
            nc.vector.tensor_tensor(out=o768[:], in0=o768[:], in1=gb[:], op=ALU.mult)
            nc.vector.tensor_tensor(out=o768[:], in0=o768[:], in1=bb[:], op=ALU.add)
            if zero_col is not None:
                nc.vector.tensor_scalar(o768[:], o768[:], zero_col, None, ALU.mult)

        # ---- dense pass ----
        for g in range(KJ // DG):
            j0 = g * DG
            wcols = slice(j0 * 8, (j0 + DG) * 8)
            cgt = epool.tile([128, DG, HID], F32, tag="C", bufs=1)
            nc.gpsimd.dma_gather(
                cgt[:],
                ctab[:], cidx16[:, wcols], DG * 128, DG * 128, HID,
            )
            pgt = epool.tile([128, DG, HID], F32, tag="P", bufs=1)
            nc.gpsimd.dma_gather(
                pgt[:],
                pose[:], pos16[:, wcols], DG * 128, DG * 128, HID,
            )
            og = epool.tile([128, DG, HID], F32, tag="O", bufs=1)
            for jj in range(DG):
                j = j0 + jj
                vt = spool.tile([128, HID], F32, tag="vtmp", bufs=2)
                nc.scalar.activation(
                    vt[:], vwb[:], ACTF.Copy, bias=0.0, scale=mval_c[:, j : j + 1]
                )
                e_ps = ppool.tile([128, HID], F32, tag="smps", bufs=2)
                for lo, hi in ((0, 512), (512, 768)):
                    nc.tensor.matmul(
                        out=e_ps[:, lo:hi], lhsT=ident[:], rhs=cgt[:, jj, lo:hi],
                        start=True, stop=False, skip_group_check=True,
                    )
                    nc.tensor.matmul(
                        out=e_ps[:, lo:hi], lhsT=ident[:], rhs=pgt[:, jj, lo:hi],
                        start=False, stop=False, skip_group_check=True,
                    )
                    nc.tensor.matmul(
                        out=e_ps[:, lo:hi], lhsT=ident[:], rhs=vt[:, lo:hi],
                        start=False, stop=True, skip_group_check=True,
                    )
                rs = spool.tile([128, 1], F32, tag="ln_rs", bufs=3)
                nbt = spool.tile([128, 1], F32, tag="ln_nb", bufs=3)
                ln_apply(
                    e_ps[:, 0:512], e_ps[:, 512:768], og[:, jj, :], rs, nbt,
                    zero_col=notm_c[:, j : j + 1],
                )
                gb_apply(og[:, jj, :], zero_col=notm_c[:, j : j + 1])
            nc.sync.dma_start(
                out=out[:N_TOK, :].rearrange("(j p) f -> p j f", p=128)[:, j0 : j0 + DG, :],
                in_=og[:],
            )

        # ---- SMILES FFN ----
        joff = 0
        for blk, nb_tok in enumerate(S_BLKS):
            kb = nb_tok // 128
            wcols = slice(joff // 16, (joff + nb_tok) // 16)
            xg = fpool.tile([128, 4, FP], F32, tag="xtok")
            nc.gpsimd.dma_gather(
                xg[:, :kb, :],
                fps[:], sfps16[:, wcols], nb_tok, nb_tok, FP,
            )
            xfm = fpool.tile([128, FP // 128, 512], BF16, tag="xfm")
            for ct in range(kb):
                for k in range(FP // 128):
                    tp = ppool.tile([128, 128], F32, tag="tp")
                    nc.tensor.transpose(
                        out=tp[:], in_=xg[:, ct, k * 128 : (k + 1) * 128], identity=ident[:]
                    )
                    dst = xfm[:, k, ct * 128 : (ct + 1) * 128]
                    if (ct * 6 + k) % 2 == 0:
                        nc.vector.tensor_copy(dst, tp[:])
                    else:
                        nc.scalar.copy(dst, tp[:])

            hid = fpool.tile([128, H4 // 128, 512], BF16, tag="hid")
            for m in range(H4 // 128):
                ph = ppool.tile([128, 512], F32, tag="mm", bufs=2)
                for k in range(FP // 128):
                    nc.tensor.matmul(
                        out=ph[:, :nb_tok],
                        lhsT=w1[:, k, m * 128 : (m + 1) * 128],
                        rhs=xfm[:, k, :nb_tok],
                        start=(k == 0),
                        stop=(k == FP // 128 - 1),
                    )
                if m % 2 == 0:
                    nc.scalar.activation(
                        hid[:, m, :nb_tok], ph[:, :nb_tok], ACTF.Relu,
                        bias=b1[:, m : m + 1], scale=1.0,
                    )
                else:
                    nc.vector.tensor_scalar(
                        hid[:, m, :nb_tok], ph[:, :nb_tok], b1[:, m : m + 1], 0.0,
                        ALU.add, ALU.max,
                    )

            ofm = fpool.tile([128, HID // 128, 512], F32, tag="ofm")
            for m2 in range(HID // 128):
                po = ppool.tile([128, 512], F32, tag="mm", bufs=2)
                for k2 in range(H4 // 128):
                    nc.tensor.matmul(
                        out=po[:, :nb_tok],
                        lhsT=w2[:, k2, m2 * 128 : (m2 + 1) * 128],
                        rhs=hid[:, k2, :nb_tok],
                        start=(k2 == 0),
                        stop=(k2 == H4 // 128 - 1),
                    )
                nc.scalar.activation(
                    ofm[:, m2, :nb_tok], po[:, :nb_tok], ACTF.Identity,
                    bias=b2[:, m2 : m2 + 1], scale=1.0,
                )

            psmb = fpool.tile([128, 4, HID], F32, tag="xtok")
            nc.gpsimd.dma_gather(
                psmb[:, :kb, :],
                pose[:], psm16[:, wcols], nb_tok, nb_tok, HID,
            )
            fo = epool.tile([128, 4, HID], F32, tag="O", bufs=1)
            for ct in range(kb):
                eps_ps = ppool.tile([128, HID], F32, tag="smps", bufs=2)
                for m2 in range(HID // 128):
                    tp2 = ppool.tile([128, 128], F32, tag="tp")
                    nc.tensor.transpose(
                        out=tp2[:], in_=ofm[:, m2, ct * 128 : (ct + 1) * 128],
                        identity=ident[:],
                    )
                    nc.vector.tensor_copy(eps_ps[:, m2 * 128 : (m2 + 1) * 128], tp2[:])
                nc.tensor.matmul(
                    out=eps_ps[:, 0:512], lhsT=ident[:], rhs=psmb[:, ct, 0:512],
                    start=False, stop=True, skip_group_check=True,
                )
                nc.tensor.matmul(
                    out=eps_ps[:, 512:768], lhsT=ident[:], rhs=psmb[:, ct, 512:768],
                    start=False, stop=True, skip_group_check=True,
                )
                rs = spool.tile([128, 1], F32, tag="ln_rs", bufs=3)
                nbt = spool.tile([128, 1], F32, tag="ln_nb", bufs=3)
                ln_apply(eps_ps[:, 0:512], eps_ps[:, 512:768], fo[:, ct, :], rs, nbt)
                gb_apply(fo[:, ct, :])
            nc.gpsimd.dma_scatter_add(
                out[:],
                fo[:, :kb, :],
                dest16[:, wcols],
                nb_tok, nb_tok, HID,
            )
            joff += nb_tok

    nc.compile()
    return nc


_CACHE = {}


def _get_program(skip_gb):
    if skip_gb not in _CACHE:
        _CACHE[skip_gb] = build_program(skip_gb)
    return _CACHE[skip_gb]


def _host_constants():
    import ml_dtypes

    ident = np.eye(128, dtype=np.float32)
    identbf = ident.astype(ml_dtypes.bfloat16)
    lexclt = np.triu(np.ones((128, 128), np.float32), 1)
    ones_col = np.ones((128, 1), np.float32)
    ones_row = np.ones((1, 128), np.float32)
    iota_c = (np.arange(KJ)[None, :] * 128 + np.arange(128)[:, None]).astype(np.int32)
    return {
        "ident": ident, "identbf": identbf, "lexclt": lexclt,
        "ones_col": ones_col, "ones_row": ones_row, "iota_c": iota_c,
    }


def kernel(**inputs):
    fps = np.ascontiguousarray(np.asarray(inputs["SMILES_fps"], np.float32).reshape(B, S, FP))
    wtok = np.asarray(inputs["word_tokens_ref"]).astype(np.int32).reshape(B, S)
    vals = np.asarray(inputs["values_ref"], np.float32).reshape(B, S)
    ttyp = np.asarray(inputs["token_type_ids"]).astype(np.int32).reshape(B, S)
    posi = np.asarray(inputs["position_ids"]).astype(np.int32).reshape(B, S)

    ln_g = np.asarray(inputs["ln_g"], np.float32)
    ln_b = np.asarray(inputs["ln_b"], np.float32)
    skip_gb = bool(np.all(ln_g == 1.0) and np.all(ln_b == 0.0))
    nc = _get_program(skip_gb)

    shared = {
        "fc1_w": np.asarray(inputs["fc1_w"], np.float32),
        "fc1_b": np.asarray(inputs["fc1_b"], np.float32),
        "fc2_w": np.asarray(inputs["fc2_w"], np.float32),
        "fc2_b": np.asarray(inputs["fc2_b"], np.float32),
        "prop": np.asarray(inputs["prop_emb"], np.float32),
        "val_w": np.asarray(inputs["val_w"], np.float32),
        "val_b": np.asarray(inputs["val_b"], np.float32),
        "pose": np.asarray(inputs["pos_emb"], np.float32),
        "typee": np.asarray(inputs["type_emb"], np.float32),
        "ln_g": ln_g, "ln_b": ln_b,
    }
    shared.update(_host_constants())

    in_maps = []
    for c in range(N_CORES):
        b0, b1 = c * B_LOC, (c + 1) * B_LOC
        n_sm = int((ttyp[b0:b1] == 1).sum())
        assert n_sm <= CAP_S, f"smiles count {n_sm} exceeds capacity {CAP_S}"
        in_maps.append(
            dict(
                shared,
                fps=fps[b0:b1].reshape(N_TOK, FP),
                wtok=wtok[b0:b1].reshape(N_TOK),
                vals=vals[b0:b1].reshape(N_TOK),
                ttyp=ttyp[b0:b1].reshape(N_TOK),
                posi=posi[b0:b1].reshape(N_TOK),
            )
        )

    res = bass_utils.run_bass_kernel_spmd(nc, in_maps, core_ids=list(range(N_CORES)))
    full = np.concatenate(
        [res.results[c]["out"][:N_TOK].reshape(B_LOC, S, HID) for c in range(N_CORES)],
        axis=0,
    )
    return full

